# revision 8
# baseline (speedup 1.0000x reference)
"""Trainium2 Bass kernel for single-step AttnDecoderRNN (batch=1 decode).

Strategy (8-way tensor parallel, one NEFF, SPMD):
  - embedding gather happens on host (only the one needed row is shipped)
  - attention (attn_W, encoder_outputs) replicated: every core computes the
    full 512 attn weights and the full attended context (cheap: ~9MB)
  - attn_combine row-sharded over H  -> x_c [256]     -> AllGather -> x [2048]
  - GRU (W_ih, W_hh) row-sharded over gates/H -> h_c [256] -> AllGather -> h
  - out_W row-sharded over V: each core computes 6250 logits + local sum-exp
    -> AllGather of the 8 partial sums -> local log-softmax normalization
  - biases folded into the matmuls as extra contraction rows (rank-1 update
    with a one-hot stationary operand), so they cost ~nothing

All matvecs y = A @ v run on the PE as out[1, n] += lhsT.T @ rhs with
lhsT = v k-slice [128, 1] (stationary) and rhs = A.T tile [128, n<=512]
(moving), with A.T pre-transposed on the host so every DMA is contiguous.
Vectors produced in free-layout [1, N] are converted to partition-layout
[128, N/128] via a PE transpose against an identity matrix.
"""

import numpy as np
import ml_dtypes

import concourse.bacc as bacc
import concourse.mybir as mybir
import concourse.tile as tile
from concourse import masks
from concourse.bass_utils import run_bass_kernel_spmd

# ---------------------------------------------------------------- constants
V, E, H, L = 50000, 300, 2048, 512
EH = E + H                      # 2348
KP = 2432                       # EH padded to 19*128 (incl. bias row at 2348)
NK = KP // 128                  # 19
NCORES = 8
VS = V // NCORES                # 6250 logits per core
HS = H // NCORES                # 256 hidden per core
GS = 3 * HS                     # 768 gate rows per core
F32 = mybir.dt.float32
BF16 = mybir.dt.bfloat16

# dtype knobs per weight group (host cast + device compute dtype)
import os as _os
_KDT = _os.environ.get("KERNEL_DTYPES", "ffff")  # attn, comb, gru, out
DT_ATTN = BF16 if _KDT[0] == "b" else F32
DT_COMB = BF16 if _KDT[1] == "b" else F32
DT_GRU = BF16 if _KDT[2] == "b" else F32
DT_OUT = BF16 if _KDT[3] == "b" else F32

_NPDT = {F32: np.float32, BF16: ml_dtypes.bfloat16}

OUT_CHUNKS = [(j * 512, min(512, VS - j * 512)) for j in range((VS + 511) // 512)]


def _np(dt):
    return _NPDT[dt]


# ---------------------------------------------------------------- device IR
def build_nc():
    nc = bacc.Bacc(trn_type="TRN2", num_devices=NCORES, debug=False)

    def din(name, shape, dt):
        return nc.dram_tensor(name, shape, dt, kind="ExternalInput").ap()

    cat1_p = din("cat1_p", [128, NK], DT_ATTN)
    emb1_p = din("emb1_p", [128, 3], DT_COMB)
    h0g_p = din("h0g_p", [128, 16], DT_GRU)
    one_g = din("one_g", [128, 1], DT_GRU)
    one_o = din("one_o", [128, 1], DT_OUT)
    h0f = din("h0f", [1, HS], F32)
    attn_WT = din("attn_WT", [KP, L], DT_ATTN)
    enc = din("enc", [L, H], DT_ATTN)
    comb_WT = din("comb_WT", [KP, HS], DT_COMB)
    wih_T = din("wih_T", [H, GS], DT_GRU)
    whh_T = din("whh_T", [H, GS], DT_GRU)
    gib = din("gib", [1, GS], DT_GRU)
    ghb = din("ghb", [1, GS], DT_GRU)
    outWT = din("outWT", [H, VS], DT_OUT)
    outb = din("outb", [1, VS], DT_OUT)

    logits_out = nc.dram_tensor("logits_out", [1, VS], F32, kind="ExternalOutput").ap()
    hidden_out = nc.dram_tensor("hidden_out", [1, H], F32, kind="ExternalOutput").ap()
    attnw_out = nc.dram_tensor("attnw_out", [1, L], F32, kind="ExternalOutput").ap()
    if _os.environ.get("KERNEL_DEBUG"):
        dbg_cat2 = nc.dram_tensor("dbg_cat2", [128, NK], DT_COMB,
                                  kind="ExternalOutput").ap()
        dbg_x = nc.dram_tensor("dbg_x", [1, H], F32, kind="ExternalOutput").ap()

    with tile.TileContext(nc) as tc:
        _body(nc, tc, locals())
    nc.compile()
    return nc


def _body(nc, tc, t):
    MM = nc.tensor.matmul
    DMA = nc.sync.dma_start
    import contextlib
    ctx = contextlib.ExitStack()
    with ctx:
        const = ctx.enter_context(tc.tile_pool(name="const", bufs=1))
        wpool = ctx.enter_context(tc.tile_pool(name="wpool", bufs=3))
        encp = ctx.enter_context(tc.tile_pool(name="encp", bufs=2))
        owtp = ctx.enter_context(tc.tile_pool(name="owtp", bufs=18))
        fpool = ctx.enter_context(tc.tile_pool(name="fpool", bufs=2))
        ps_s = ctx.enter_context(tc.tile_pool(name="ps_s", bufs=2, space="PSUM"))
        ps_tr = ctx.enter_context(tc.tile_pool(name="ps_tr", bufs=2, space="PSUM"))
        ps_o = ctx.enter_context(tc.tile_pool(name="ps_o", bufs=4, space="PSUM"))
        dram = ctx.enter_context(tc.tile_pool(name="dram", bufs=1, space="DRAM"))

        # ---- constants / small inputs
        ident = const.tile([128, 128], F32)
        masks.make_identity(nc, ident[:])
        ca1 = const.tile([128, NK], DT_ATTN)
        DMA(ca1[:], t["cat1_p"])
        emb1 = const.tile([128, 3], DT_COMB)
        DMA(emb1[:], t["emb1_p"])
        h0g = const.tile([128, 16], DT_GRU)
        DMA(h0g[:], t["h0g_p"])
        oneg = const.tile([128, 1], DT_GRU)
        DMA(oneg[:], t["one_g"])
        oneo = const.tile([128, 1], DT_OUT)
        DMA(oneo[:], t["one_o"])
        h0f_t = const.tile([1, HS], F32)
        DMA(h0f_t[:], t["h0f"])
        gib_t = const.tile([1, GS], DT_GRU)
        DMA(gib_t[:], t["gib"])
        ghb_t = const.tile([1, GS], DT_GRU)
        DMA(ghb_t[:], t["ghb"])

        # ---- attention logits: al[1, 512] = cat1 @ attn_W.T (+attn_b via row 2348)
        psum_al = ps_s.tile([1, 512], F32, tag="pss")
        for k in range(NK):
            awt = wpool.tile([128, L], DT_ATTN, tag="awt", name=f"awt{k}")
            DMA(awt[:], t["attn_WT"][k * 128:(k + 1) * 128, :])
            MM(psum_al[:], lhsT=ca1[:, k:k + 1], rhs=awt[:],
               start=(k == 0), stop=(k == NK - 1))

        # ---- softmax over 512 on partition 0
        mx = const.tile([1, 1], F32)
        nc.vector.reduce_max(mx[:], psum_al[:], axis=mybir.AxisListType.X)
        negm = const.tile([1, 1], F32)
        nc.vector.tensor_scalar_mul(negm[:], mx[:], -1.0)
        e_sb = const.tile([1, 512], F32)
        s1 = const.tile([1, 1], F32)
        nc.scalar.activation(e_sb[:], psum_al[:], mybir.ActivationFunctionType.Exp,
                             bias=negm[:], scale=1.0, accum_out=s1[:])
        rs = const.tile([1, 1], F32)
        nc.vector.reciprocal(rs[:], s1[:])
        aw_sb = const.tile([1, 512], F32)
        nc.vector.tensor_scalar_mul(aw_sb[:], e_sb[:], rs[:])
        DMA(t["attnw_out"], aw_sb[:])

        # ---- attn weights to partition layout [128, 4]
        aw4 = const.tile([4, 128], F32)
        for i in range(4):
            DMA(aw4[i:i + 1, :], aw_sb[:, i * 128:(i + 1) * 128])
        ps_awp = ps_tr.tile([128, 4], F32, tag="ptr")
        nc.tensor.transpose(ps_awp[:], aw4[:], ident[0:4, 0:4])
        wp = const.tile([128, 4], DT_ATTN)
        nc.vector.tensor_copy(wp[:], ps_awp[:])

        # ---- attended context directly in partition layout [128, 16]
        # NOTE: accumulation groups within one PSUM tile must be contiguous
        # (start=True clears bank-wide), so loop j-outer / k-inner.
        ps_aa = ps_tr.tile([128, 16], F32, tag="ptr")
        encks = []
        for k in range(4):
            enck = encp.tile([128, H], DT_ATTN, tag=f"enck{k}", name=f"enck{k}")
            DMA(enck[:], t["enc"][k * 128:(k + 1) * 128, :])
            encks.append(enck)
        for j in range(16):
            for k in range(4):
                MM(ps_aa[:, j:j + 1], lhsT=encks[k][:, j * 128:(j + 1) * 128],
                   rhs=wp[:, k:k + 1], start=(k == 0), stop=(k == 3))

        cat2 = const.tile([128, NK], DT_COMB)
        nc.vector.tensor_copy(cat2[:, 0:16], ps_aa[:])
        nc.scalar.copy(cat2[:, 16:19], emb1[:])
        if "dbg_cat2" in t:
            DMA(t["dbg_cat2"], cat2[:])

        # ---- attn_combine row-shard: x_c[1, 256] = relu(cat2 @ comb_W_c.T + b)
        psum_x = ps_s.tile([1, 512], F32, tag="pss")
        for k in range(NK):
            cwt = wpool.tile([128, HS], DT_COMB, tag="cwt", name=f"cwt{k}")
            DMA(cwt[:], t["comb_WT"][k * 128:(k + 1) * 128, :])
            MM(psum_x[:, 0:HS], lhsT=cat2[:, k:k + 1], rhs=cwt[:],
               start=(k == 0), stop=(k == NK - 1))
        x_sb = const.tile([1, HS], F32)
        nc.scalar.activation(x_sb[:], psum_x[:, 0:HS],
                             mybir.ActivationFunctionType.Relu)

        # ---- AllGather x -> [2048] -> partition layout
        xin_d = dram.tile([1, HS], F32)
        xg_d = dram.tile([1, H], F32)
        DMA(xin_d[:], x_sb[:])
        nc.gpsimd.collective_compute(
            "AllGather", mybir.AluOpType.bypass,
            replica_groups=[list(range(NCORES))],
            ins=[xin_d.opt()], outs=[xg_d.opt()])
        if "dbg_x" in t:
            DMA(t["dbg_x"], xg_d[:])
        xf = const.tile([16, 128], F32)
        DMA(xf[:], xg_d[:].rearrange("a (b c) -> (a b) c", c=128))
        ps_xp = ps_tr.tile([128, 16], F32, tag="ptr")
        nc.tensor.transpose(ps_xp[:], xf[:], ident[0:16, 0:16])
        xp = const.tile([128, 16], DT_GRU)
        nc.vector.tensor_copy(xp[:], ps_xp[:])

        # ---- GRU row-shard: gates r,z,n for this core's 256 hidden slots
        # gh = h0 @ W_hh_c.T + b_hh_c ; gi = x @ W_ih_c.T + b_ih_c
        ps_gh_a = ps_s.tile([1, 512], F32, tag="pss")
        ps_gh_b = ps_s.tile([1, 512], F32, tag="pss")
        MM(ps_gh_a[:], lhsT=oneg[0:1, 0:1], rhs=ghb_t[:, 0:512],
           start=True, stop=False)
        MM(ps_gh_b[:, 0:HS], lhsT=oneg[0:1, 0:1], rhs=ghb_t[:, 512:768],
           start=True, stop=False)
        for k in range(16):
            whk = wpool.tile([128, GS], DT_GRU, tag="gwt", name=f"whk{k}")
            DMA(whk[:], t["whh_T"][k * 128:(k + 1) * 128, :])
            MM(ps_gh_a[:], lhsT=h0g[:, k:k + 1], rhs=whk[:, 0:512],
               start=False, stop=(k == 15))
            MM(ps_gh_b[:, 0:HS], lhsT=h0g[:, k:k + 1], rhs=whk[:, 512:768],
               start=False, stop=(k == 15))
        gha = const.tile([1, 512], F32)
        nc.scalar.copy(gha[:], ps_gh_a[:])
        ghb_sb = const.tile([1, HS], F32)
        nc.scalar.copy(ghb_sb[:], ps_gh_b[:, 0:HS])

        ps_gi_a = ps_s.tile([1, 512], F32, tag="pss")
        ps_gi_b = ps_s.tile([1, 512], F32, tag="pss")
        MM(ps_gi_a[:], lhsT=oneg[0:1, 0:1], rhs=gib_t[:, 0:512],
           start=True, stop=False)
        MM(ps_gi_b[:, 0:HS], lhsT=oneg[0:1, 0:1], rhs=gib_t[:, 512:768],
           start=True, stop=False)
        for k in range(16):
            wik = wpool.tile([128, GS], DT_GRU, tag="gwt", name=f"wik{k}")
            DMA(wik[:], t["wih_T"][k * 128:(k + 1) * 128, :])
            MM(ps_gi_a[:], lhsT=xp[:, k:k + 1], rhs=wik[:, 0:512],
               start=False, stop=(k == 15))
            MM(ps_gi_b[:, 0:HS], lhsT=xp[:, k:k + 1], rhs=wik[:, 512:768],
               start=False, stop=(k == 15))

        rzpre = const.tile([1, 512], F32)
        nc.vector.tensor_add(rzpre[:], ps_gi_a[:], gha[:])
        rz = const.tile([1, 512], F32)
        nc.scalar.activation(rz[:], rzpre[:], mybir.ActivationFunctionType.Sigmoid)
        rhn = const.tile([1, HS], F32)
        nc.vector.tensor_mul(rhn[:], rz[:, 0:HS], ghb_sb[:])
        npre = const.tile([1, HS], F32)
        nc.vector.tensor_add(npre[:], ps_gi_b[:, 0:HS], rhn[:])
        n_sb = const.tile([1, HS], F32)
        nc.scalar.activation(n_sb[:], npre[:], mybir.ActivationFunctionType.Tanh)
        dd = const.tile([1, HS], F32)
        nc.vector.tensor_sub(dd[:], h0f_t[:], n_sb[:])
        zd = const.tile([1, HS], F32)
        nc.vector.tensor_mul(zd[:], rz[:, HS:2 * HS], dd[:])
        hnew = const.tile([1, HS], F32)
        nc.vector.tensor_add(hnew[:], n_sb[:], zd[:])

        # ---- AllGather h -> [2048]; emit hidden output; partition layout
        hin_d = dram.tile([1, HS], F32)
        hg_d = dram.tile([1, H], F32)
        DMA(hin_d[:], hnew[:])
        nc.gpsimd.collective_compute(
            "AllGather", mybir.AluOpType.bypass,
            replica_groups=[list(range(NCORES))],
            ins=[hin_d.opt()], outs=[hg_d.opt()])
        DMA(t["hidden_out"], hg_d[:])
        hf = const.tile([16, 128], F32)
        DMA(hf[:], hg_d[:].rearrange("a (b c) -> (a b) c", c=128))
        ps_hp = ps_tr.tile([128, 16], F32, tag="ptr")
        nc.tensor.transpose(ps_hp[:], hf[:], ident[0:16, 0:16])
        hp = const.tile([128, 16], DT_OUT)
        nc.vector.tensor_copy(hp[:], ps_hp[:])

        # ---- output projection row-shard: logits_c[6250] = h @ out_W_c.T + b_c
        ssum = const.tile([1, len(OUT_CHUNKS)], F32)
        lgr_d = dram.tile([1, VS], F32)
        for jp in range((len(OUT_CHUNKS) + 1) // 2):
            tiles_k = []
            w_pair = min(1024, VS - jp * 1024)
            for k in range(16):
                owt = owtp.tile([128, 1024], DT_OUT, tag="owt",
                                name=f"owt{jp}_{k}")
                DMA(owt[:, 0:w_pair],
                    t["outWT"][k * 128:(k + 1) * 128,
                               jp * 1024:jp * 1024 + w_pair])
                tiles_k.append(owt)
            for jj in range(2):
                j = jp * 2 + jj
                if j >= len(OUT_CHUNKS):
                    break
                off, w = OUT_CHUNKS[j]
                ps = ps_o.tile([1, 512], F32, tag="po", name=f"po{j}")
                ob = fpool.tile([1, 512], DT_OUT, tag="ob", name=f"ob{j}")
                DMA(ob[:, 0:w], t["outb"][:, off:off + w])
                MM(ps[:, 0:w], lhsT=oneo[0:1, 0:1], rhs=ob[:, 0:w],
                   start=True, stop=False)
                for k in range(16):
                    MM(ps[:, 0:w], lhsT=hp[:, k:k + 1],
                       rhs=tiles_k[k][:, jj * 512:jj * 512 + w],
                       start=False, stop=(k == 15))
                raw = fpool.tile([1, 512], F32, tag="raw", name=f"raw{j}", bufs=2)
                nc.scalar.copy(raw[:, 0:w], ps[:, 0:w])
                DMA(lgr_d[:, off:off + w], raw[:, 0:w])
                esc = fpool.tile([1, 512], F32, tag="esc", name=f"esc{j}")
                nc.scalar.activation(esc[:, 0:w], ps[:, 0:w],
                                     mybir.ActivationFunctionType.Exp,
                                     accum_out=ssum[:, j:j + 1])

        # ---- global log-sum-exp via AllGather of the 8 local sums
        sl = const.tile([1, 1], F32)
        nc.vector.reduce_sum(sl[:], ssum[:], axis=mybir.AxisListType.X)
        sin_d = dram.tile([1, 1], F32)
        sg_d = dram.tile([1, NCORES], F32)
        DMA(sin_d[:], sl[:])
        nc.gpsimd.collective_compute(
            "AllGather", mybir.AluOpType.bypass,
            replica_groups=[list(range(NCORES))],
            ins=[sin_d.opt()], outs=[sg_d.opt()])
        s8 = const.tile([1, NCORES], F32)
        DMA(s8[:], sg_d[:])
        st = const.tile([1, 1], F32)
        nc.vector.reduce_sum(st[:], s8[:], axis=mybir.AxisListType.X)
        logs = const.tile([1, 1], F32)
        nc.scalar.activation(logs[:], st[:], mybir.ActivationFunctionType.Ln)
        negls = const.tile([1, 1], F32)
        nc.vector.tensor_scalar_mul(negls[:], logs[:], -1.0)

        # ---- final normalize: out = logits - logS (alternate DVE/ACT)
        for j, (off, w) in enumerate(OUT_CHUNKS):
            lgin = fpool.tile([1, 512], F32, tag="lgin", name=f"lgin{j}", bufs=3)
            DMA(lgin[:, 0:w], lgr_d[:, off:off + w])
            outc = fpool.tile([1, 512], F32, tag="outc", name=f"outc{j}", bufs=3)
            if j % 2 == 0:
                nc.vector.tensor_scalar_add(outc[:, 0:w], lgin[:, 0:w], negls[:])
            else:
                nc.scalar.activation(outc[:, 0:w], lgin[:, 0:w],
                                     mybir.ActivationFunctionType.Identity,
                                     bias=negls[:], scale=1.0)
            DMA(t["logits_out"][:, off:off + w], outc[:, 0:w])


# ---------------------------------------------------------------- host prep
def shard_inputs(input, hidden, encoder_outputs, emb, attn_W, attn_b,
                 comb_W, comb_b, W_ih, W_hh, b_ih, b_hh, out_W, out_b):
    """Build the 8 per-core input maps (numpy)."""
    idx = int(np.asarray(input).reshape(-1)[0])
    embedded = np.asarray(emb[idx], dtype=np.float32)          # [300]
    h0 = np.asarray(hidden, dtype=np.float32).reshape(H)       # [2048]
    attn_W = np.asarray(attn_W, dtype=np.float32)
    attn_b = np.asarray(attn_b, dtype=np.float32)
    comb_W = np.asarray(comb_W, dtype=np.float32)
    comb_b = np.asarray(comb_b, dtype=np.float32)
    W_ih = np.asarray(W_ih, dtype=np.float32)
    W_hh = np.asarray(W_hh, dtype=np.float32)
    b_ih = np.asarray(b_ih, dtype=np.float32)
    b_hh = np.asarray(b_hh, dtype=np.float32)
    out_W = np.asarray(out_W, dtype=np.float32)
    out_b = np.asarray(out_b, dtype=np.float32)
    enc = np.asarray(encoder_outputs, dtype=np.float32)

    # cat1 (reordered): [h0; embedded; 1.0; zeros] in partition layout
    cat1 = np.zeros(KP, dtype=np.float32)
    cat1[0:H] = h0
    cat1[H:H + E] = embedded
    cat1[EH] = 1.0
    cat1_p = np.ascontiguousarray(cat1.reshape(NK, 128).T, dtype=_np(DT_ATTN))
    emb1_p = np.ascontiguousarray(
        cat1[H:].reshape(3, 128).T, dtype=_np(DT_COMB))
    h0g_p = np.ascontiguousarray(h0.reshape(16, 128).T, dtype=_np(DT_GRU))
    one = np.zeros((128, 1), dtype=np.float32)
    one[0, 0] = 1.0

    # attn_W columns reordered to [h-part; e-part], bias row appended
    awt = np.zeros((KP, L), dtype=np.float32)
    awt[0:H] = attn_W[:, E:EH].T
    awt[H:EH] = attn_W[:, 0:E].T
    awt[EH] = attn_b
    awt = awt.astype(_np(DT_ATTN))

    enc_c = np.ascontiguousarray(enc, dtype=_np(DT_ATTN))

    per_core = []
    for c in range(NCORES):
        rows = slice(c * HS, (c + 1) * HS)
        cwt = np.zeros((KP, HS), dtype=np.float32)
        cwt[0:H] = comb_W[rows, E:EH].T
        cwt[H:EH] = comb_W[rows, 0:E].T
        cwt[EH] = comb_b[rows]
        grows = np.concatenate(
            [np.arange(g * H + c * HS, g * H + (c + 1) * HS) for g in range(3)])
        wih_t = np.ascontiguousarray(W_ih[grows].T, dtype=_np(DT_GRU))
        whh_t = np.ascontiguousarray(W_hh[grows].T, dtype=_np(DT_GRU))
        vrows = slice(c * VS, (c + 1) * VS)
        owt = np.ascontiguousarray(out_W[vrows].T, dtype=_np(DT_OUT))
        per_core.append({
            "cat1_p": cat1_p,
            "emb1_p": emb1_p,
            "h0g_p": h0g_p,
            "one_g": one.astype(_np(DT_GRU)),
            "one_o": one.astype(_np(DT_OUT)),
            "h0f": h0[rows].reshape(1, HS).copy(),
            "attn_WT": awt,
            "enc": enc_c,
            "comb_WT": cwt.astype(_np(DT_COMB)),
            "wih_T": wih_t,
            "whh_T": whh_t,
            "gib": b_ih[grows].reshape(1, GS).astype(_np(DT_GRU)),
            "ghb": b_hh[grows].reshape(1, GS).astype(_np(DT_GRU)),
            "outWT": owt,
            "outb": out_b[vrows].reshape(1, VS).astype(_np(DT_OUT)),
        })
    return per_core


# ---------------------------------------------------------------- entry
_CACHED_NC = None
LAST_RESULT = None


def kernel(**inputs):
    global _CACHED_NC, LAST_RESULT
    try:
        import axon_profile_shim
        axon_profile_shim.install()
    except Exception:
        pass
    if _CACHED_NC is None:
        _CACHED_NC = build_nc()
    in_maps = shard_inputs(**inputs)
    trace = bool(int(__import__("os").environ.get("KERNEL_TRACE", "0")))
    res = run_bass_kernel_spmd(
        _CACHED_NC, in_maps, core_ids=list(range(NCORES)), trace=trace)
    LAST_RESULT = res
    logits = np.concatenate(
        [res.results[c]["logits_out"] for c in range(NCORES)], axis=1)
    hidden_new = res.results[0]["hidden_out"].reshape(1, 1, H)
    attn_weights = res.results[0]["attnw_out"].reshape(1, L)
    return (logits.astype(np.float32),
            hidden_new.astype(np.float32),
            attn_weights.astype(np.float32))


# revision 18
# speedup vs baseline: 1.1806x; 1.1806x over previous
"""Trainium2 Bass kernel for single-step AttnDecoderRNN (batch=1 decode).

Strategy (8-way tensor parallel, one NEFF, SPMD):
  - embedding gather happens on host (only the one needed row is shipped)
  - attention (attn_W, encoder_outputs) replicated: every core computes the
    full 512 attn weights and the full attended context (cheap: ~9MB)
  - attn_combine row-sharded over H  -> x_c [256]     -> AllGather -> x [2048]
  - GRU (W_ih, W_hh) row-sharded over gates/H -> h_c [256] -> AllGather -> h
  - out_W row-sharded over V: each core computes 6250 logits + local sum-exp
    -> AllGather of the 8 partial sums -> local log-softmax normalization
  - biases folded into the matmuls as extra contraction rows (rank-1 update
    with a one-hot stationary operand), so they cost ~nothing

All matvecs y = A @ v run on the PE as out[1, n] += lhsT.T @ rhs with
lhsT = v k-slice [128, 1] (stationary) and rhs = A.T tile [128, n<=512]
(moving), with A.T pre-transposed on the host so every DMA is contiguous.
Vectors produced in free-layout [1, N] are converted to partition-layout
[128, N/128] via a PE transpose against an identity matrix.
"""

import numpy as np
import ml_dtypes

import concourse.bacc as bacc
import concourse.mybir as mybir
import concourse.tile as tile
from concourse import masks
from concourse.bass_utils import run_bass_kernel_spmd

# ---------------------------------------------------------------- constants
V, E, H, L = 50000, 300, 2048, 512
EH = E + H                      # 2348
KP = 2432                       # EH padded to 19*128 (incl. bias row at 2348)
NK = KP // 128                  # 19
NCORES = 8
VS = V // NCORES                # 6250 logits per core
HS = H // NCORES                # 256 hidden per core
GS = 3 * HS                     # 768 gate rows per core
F32 = mybir.dt.float32
BF16 = mybir.dt.bfloat16

# dtype knobs per weight group (host cast + device compute dtype)
import os as _os
_KDT = _os.environ.get("KERNEL_DTYPES", "ffff")  # attn, comb, gru, out
DT_ATTN = BF16 if _KDT[0] == "b" else F32
DT_COMB = BF16 if _KDT[1] == "b" else F32
DT_GRU = BF16 if _KDT[2] == "b" else F32
DT_OUT = BF16 if _KDT[3] == "b" else F32

_NPDT = {F32: np.float32, BF16: ml_dtypes.bfloat16}

OUT_CHUNKS = [(j * 512, min(512, VS - j * 512)) for j in range((VS + 511) // 512)]


def _np(dt):
    return _NPDT[dt]


# ---------------------------------------------------------------- device IR
def build_nc():
    nc = bacc.Bacc(trn_type="TRN2", num_devices=NCORES, debug=False)

    def din(name, shape, dt):
        return nc.dram_tensor(name, shape, dt, kind="ExternalInput").ap()

    cat1_p = din("cat1_p", [128, NK], DT_ATTN)
    emb1_p = din("emb1_p", [128, 3], DT_COMB)
    h0g_p = din("h0g_p", [128, 16], DT_GRU)
    one_g = din("one_g", [128, 1], DT_GRU)
    one_o = din("one_o", [128, 1], DT_OUT)
    h0f = din("h0f", [1, HS], F32)
    attn_WT = din("attn_WT", [KP, L], DT_ATTN)
    enc = din("enc", [L, H], DT_ATTN)
    comb_WT = din("comb_WT", [KP, HS], DT_COMB)
    wih_T = din("wih_T", [H, GS], DT_GRU)
    whh_T = din("whh_T", [H, GS], DT_GRU)
    gib = din("gib", [1, GS], DT_GRU)
    ghb = din("ghb", [1, GS], DT_GRU)
    outWT = din("outWT", [H, VS], DT_OUT)
    outb = din("outb", [1, VS], DT_OUT)

    logits_out = nc.dram_tensor("logits_out", [1, VS], F32, kind="ExternalOutput").ap()
    hidden_out = nc.dram_tensor("hidden_out", [1, H], F32, kind="ExternalOutput").ap()
    attnw_out = nc.dram_tensor("attnw_out", [1, L], F32, kind="ExternalOutput").ap()
    if _os.environ.get("KERNEL_DEBUG"):
        dbg_cat2 = nc.dram_tensor("dbg_cat2", [128, NK], DT_COMB,
                                  kind="ExternalOutput").ap()
        dbg_x = nc.dram_tensor("dbg_x", [1, H], F32, kind="ExternalOutput").ap()

    with tile.TileContext(nc) as tc:
        _body(nc, tc, locals())
    nc.compile()
    return nc


def _body(nc, tc, t):
    MM = nc.tensor.matmul
    DMA = nc.sync.dma_start
    import contextlib
    ctx = contextlib.ExitStack()
    with ctx:
        const = ctx.enter_context(tc.tile_pool(name="const", bufs=1))
        bigw = ctx.enter_context(tc.tile_pool(name="bigw", bufs=2))
        owtp = ctx.enter_context(tc.tile_pool(name="owtp", bufs=20))
        fpool = ctx.enter_context(tc.tile_pool(name="fpool", bufs=2))
        ps_s = ctx.enter_context(tc.tile_pool(name="ps_s", bufs=3, space="PSUM"))
        ps_o = ctx.enter_context(tc.tile_pool(name="ps_o", bufs=5, space="PSUM"))
        dram = ctx.enter_context(tc.tile_pool(name="dram", bufs=1, space="DRAM"))

        def load_weight(name, src, kn, width, dt, halves=2):
            """One SBUF tile [128, kn*width]; col-block k = src[k*128:(k+1)*128, :].
            DMA'd in `halves` pieces for pipelining (subtile deps)."""
            w = bigw.tile([128, kn * width], dt, tag="bigw", name=name)
            srcv = src.rearrange("(k p) n -> p k n", p=128)
            step = (kn + halves - 1) // halves
            for h0 in range(0, kn, step):
                h1 = min(h0 + step, kn)
                DMA(w[:, h0 * width:h1 * width], srcv[:, h0:h1, :])
            return w

        # ---- constants / small inputs
        ident = const.tile([128, 128], F32)
        masks.make_identity(nc, ident[:])
        ca1 = const.tile([128, NK], DT_ATTN)
        DMA(ca1[:], t["cat1_p"])
        emb1 = const.tile([128, 3], DT_COMB)
        DMA(emb1[:], t["emb1_p"])
        h0g = const.tile([128, 16], DT_GRU)
        DMA(h0g[:], t["h0g_p"])
        oneg = const.tile([128, 1], DT_GRU)
        DMA(oneg[:], t["one_g"])
        oneo = const.tile([128, 1], DT_OUT)
        DMA(oneo[:], t["one_o"])
        h0f_t = const.tile([1, HS], F32)
        DMA(h0f_t[:], t["h0f"])
        gib_t = const.tile([1, GS], DT_GRU)
        DMA(gib_t[:], t["gib"])
        ghb_t = const.tile([1, GS], DT_GRU)
        DMA(ghb_t[:], t["ghb"])

        # ---- attention logits: al[1, 512] = cat1 @ attn_W.T (+attn_b via row 2348)
        awt = load_weight("awt", t["attn_WT"], NK, L, DT_ATTN, halves=3)
        psum_al = ps_s.tile([1, 512], F32, tag="pss")
        for k in range(NK):
            MM(psum_al[:], lhsT=ca1[:, k:k + 1], rhs=awt[:, k * L:(k + 1) * L],
               start=(k == 0), stop=(k == NK - 1))

        # ---- softmax over 512 on partition 0
        mx = const.tile([1, 1], F32)
        nc.vector.reduce_max(mx[:], psum_al[:], axis=mybir.AxisListType.X)
        negm = const.tile([1, 1], F32)
        nc.vector.tensor_scalar_mul(negm[:], mx[:], -1.0)
        e_sb = const.tile([1, 512], F32)
        s1 = const.tile([1, 1], F32)
        nc.scalar.activation(e_sb[:], psum_al[:], mybir.ActivationFunctionType.Exp,
                             bias=negm[:], scale=1.0, accum_out=s1[:])
        rs = const.tile([1, 1], F32)
        nc.vector.reciprocal(rs[:], s1[:])
        aw_sb = const.tile([1, 512], F32)
        nc.vector.tensor_scalar_mul(aw_sb[:], e_sb[:], rs[:])
        DMA(t["attnw_out"], aw_sb[:])

        # ---- attn weights to partition layout [128, 4]
        aw4 = const.tile([4, 128], F32)
        for i in range(4):
            DMA(aw4[i:i + 1, :], aw_sb[:, i * 128:(i + 1) * 128])
        ps_awp = ps_s.tile([128, 4], F32, tag="pss")
        nc.tensor.transpose(ps_awp[:], aw4[:], ident[0:4, 0:4])
        wp = const.tile([128, 4], DT_ATTN)
        nc.vector.tensor_copy(wp[:], ps_awp[:])

        # ---- attended context directly in partition layout [128, 16]
        # NOTE: accumulation groups within one PSUM tile must be contiguous
        # (start=True clears bank-wide), so loop j-outer / k-inner.
        encw = load_weight("encw", t["enc"], 4, H, DT_ATTN, halves=2)
        ps_aa = ps_s.tile([128, 16], F32, tag="pss")
        for j in range(16):
            for k in range(4):
                MM(ps_aa[:, j:j + 1],
                   lhsT=encw[:, k * H + j * 128:k * H + (j + 1) * 128],
                   rhs=wp[:, k:k + 1], start=(k == 0), stop=(k == 3))

        cat2 = const.tile([128, NK], DT_COMB)
        nc.vector.tensor_copy(cat2[:, 0:16], ps_aa[:])
        nc.scalar.copy(cat2[:, 16:19], emb1[:])
        if "dbg_cat2" in t:
            DMA(t["dbg_cat2"], cat2[:])

        # ---- attn_combine row-shard: x_c[1, 256] = relu(cat2 @ comb_W_c.T + b)
        cwt = load_weight("cwt", t["comb_WT"], NK, HS, DT_COMB, halves=1)
        psum_x = ps_s.tile([1, 512], F32, tag="pss")
        for k in range(NK):
            MM(psum_x[:, 0:HS], lhsT=cat2[:, k:k + 1],
               rhs=cwt[:, k * HS:(k + 1) * HS],
               start=(k == 0), stop=(k == NK - 1))
        x_sb = const.tile([1, HS], F32)
        nc.scalar.activation(x_sb[:], psum_x[:, 0:HS],
                             mybir.ActivationFunctionType.Relu)

        # ---- AllGather x -> [2048] -> partition layout
        xin_d = dram.tile([1, HS], F32)
        xg_d = dram.tile([1, H], F32)
        DMA(xin_d[:], x_sb[:])
        nc.gpsimd.collective_compute(
            "AllGather", mybir.AluOpType.bypass,
            replica_groups=[list(range(NCORES))],
            ins=[xin_d.opt()], outs=[xg_d.opt()])
        if "dbg_x" in t:
            DMA(t["dbg_x"], xg_d[:])
        xf = const.tile([16, 128], F32)
        DMA(xf[:], xg_d[:].rearrange("a (b c) -> (a b) c", c=128))
        ps_xp = ps_s.tile([128, 16], F32, tag="pss")
        nc.tensor.transpose(ps_xp[:], xf[:], ident[0:16, 0:16])
        xp = const.tile([128, 16], DT_GRU)
        nc.vector.tensor_copy(xp[:], ps_xp[:])

        # ---- GRU row-shard: gates r,z,n for this core's 256 hidden slots
        # gh = h0 @ W_hh_c.T + b_hh_c ; gi = x @ W_ih_c.T + b_ih_c
        ps_gh_a = ps_s.tile([1, 512], F32, tag="pss")
        ps_gh_b = ps_s.tile([1, 512], F32, tag="pss")
        whw = load_weight("whw", t["whh_T"], 16, GS, DT_GRU, halves=2)
        MM(ps_gh_a[:], lhsT=oneg[0:1, 0:1], rhs=ghb_t[:, 0:512],
           start=True, stop=False)
        MM(ps_gh_b[:, 0:HS], lhsT=oneg[0:1, 0:1], rhs=ghb_t[:, 512:768],
           start=True, stop=False)
        for k in range(16):
            MM(ps_gh_a[:], lhsT=h0g[:, k:k + 1], rhs=whw[:, k * GS:k * GS + 512],
               start=False, stop=(k == 15))
            MM(ps_gh_b[:, 0:HS], lhsT=h0g[:, k:k + 1],
               rhs=whw[:, k * GS + 512:(k + 1) * GS],
               start=False, stop=(k == 15))
        gha = const.tile([1, 512], F32)
        nc.scalar.copy(gha[:], ps_gh_a[:])
        ghb_sb = const.tile([1, HS], F32)
        nc.scalar.copy(ghb_sb[:], ps_gh_b[:, 0:HS])

        ps_gi_a = ps_s.tile([1, 512], F32, tag="pss")
        ps_gi_b = ps_s.tile([1, 512], F32, tag="pss")
        wiw = load_weight("wiw", t["wih_T"], 16, GS, DT_GRU, halves=2)
        MM(ps_gi_a[:], lhsT=oneg[0:1, 0:1], rhs=gib_t[:, 0:512],
           start=True, stop=False)
        MM(ps_gi_b[:, 0:HS], lhsT=oneg[0:1, 0:1], rhs=gib_t[:, 512:768],
           start=True, stop=False)
        for k in range(16):
            MM(ps_gi_a[:], lhsT=xp[:, k:k + 1], rhs=wiw[:, k * GS:k * GS + 512],
               start=False, stop=(k == 15))
            MM(ps_gi_b[:, 0:HS], lhsT=xp[:, k:k + 1],
               rhs=wiw[:, k * GS + 512:(k + 1) * GS],
               start=False, stop=(k == 15))

        rzpre = const.tile([1, 512], F32)
        nc.vector.tensor_add(rzpre[:], ps_gi_a[:], gha[:])
        rz = const.tile([1, 512], F32)
        nc.scalar.activation(rz[:], rzpre[:], mybir.ActivationFunctionType.Sigmoid)
        rhn = const.tile([1, HS], F32)
        nc.vector.tensor_mul(rhn[:], rz[:, 0:HS], ghb_sb[:])
        npre = const.tile([1, HS], F32)
        nc.vector.tensor_add(npre[:], ps_gi_b[:, 0:HS], rhn[:])
        n_sb = const.tile([1, HS], F32)
        nc.scalar.activation(n_sb[:], npre[:], mybir.ActivationFunctionType.Tanh)
        dd = const.tile([1, HS], F32)
        nc.vector.tensor_sub(dd[:], h0f_t[:], n_sb[:])
        zd = const.tile([1, HS], F32)
        nc.vector.tensor_mul(zd[:], rz[:, HS:2 * HS], dd[:])
        hnew = const.tile([1, HS], F32)
        nc.vector.tensor_add(hnew[:], n_sb[:], zd[:])

        # ---- AllGather h -> [2048]; emit hidden output; partition layout
        hin_d = dram.tile([1, HS], F32)
        hg_d = dram.tile([1, H], F32)
        DMA(hin_d[:], hnew[:])
        nc.gpsimd.collective_compute(
            "AllGather", mybir.AluOpType.bypass,
            replica_groups=[list(range(NCORES))],
            ins=[hin_d.opt()], outs=[hg_d.opt()])
        DMA(t["hidden_out"], hg_d[:])
        hf = const.tile([16, 128], F32)
        DMA(hf[:], hg_d[:].rearrange("a (b c) -> (a b) c", c=128))
        ps_hp = ps_s.tile([128, 16], F32, tag="pss")
        nc.tensor.transpose(ps_hp[:], hf[:], ident[0:16, 0:16])
        hp = const.tile([128, 16], DT_OUT)
        nc.vector.tensor_copy(hp[:], ps_hp[:])

        # ---- output projection row-shard: logits_c[6250] = h @ out_W_c.T + b_c
        # DMA granularity: [128, <=2048] per (group, k); 4 chunks of 512/group.
        ssum = const.tile([1, len(OUT_CHUNKS)], F32)
        lgr_d = dram.tile([1, VS], F32)
        GRP = 2048
        n_grp = (VS + GRP - 1) // GRP
        for g in range(n_grp):
            g0 = g * GRP
            gw = min(GRP, VS - g0)
            tiles_k = []
            for k in range(16):
                owt = owtp.tile([128, GRP], DT_OUT, tag="owt",
                                name=f"owt{g}_{k}")
                DMA(owt[:, 0:gw],
                    t["outWT"][k * 128:(k + 1) * 128, g0:g0 + gw])
                tiles_k.append(owt)
            for jj in range((gw + 511) // 512):
                j = g * 4 + jj
                off, w = OUT_CHUNKS[j]
                ps = ps_o.tile([1, 512], F32, tag="po", name=f"po{j}")
                ob = fpool.tile([1, 512], DT_OUT, tag="ob", name=f"ob{j}")
                DMA(ob[:, 0:w], t["outb"][:, off:off + w])
                MM(ps[:, 0:w], lhsT=oneo[0:1, 0:1], rhs=ob[:, 0:w],
                   start=True, stop=False)
                for k in range(16):
                    MM(ps[:, 0:w], lhsT=hp[:, k:k + 1],
                       rhs=tiles_k[k][:, jj * 512:jj * 512 + w],
                       start=False, stop=(k == 15))
                raw = fpool.tile([1, 512], F32, tag="raw", name=f"raw{j}", bufs=2)
                nc.scalar.copy(raw[:, 0:w], ps[:, 0:w])
                DMA(lgr_d[:, off:off + w], raw[:, 0:w])
                esc = fpool.tile([1, 512], F32, tag="esc", name=f"esc{j}")
                nc.scalar.activation(esc[:, 0:w], ps[:, 0:w],
                                     mybir.ActivationFunctionType.Exp,
                                     accum_out=ssum[:, j:j + 1])

        # ---- global log-sum-exp via AllGather of the 8 local sums
        sl = const.tile([1, 1], F32)
        nc.vector.reduce_sum(sl[:], ssum[:], axis=mybir.AxisListType.X)
        sin_d = dram.tile([1, 1], F32)
        sg_d = dram.tile([1, NCORES], F32)
        DMA(sin_d[:], sl[:])
        nc.gpsimd.collective_compute(
            "AllGather", mybir.AluOpType.bypass,
            replica_groups=[list(range(NCORES))],
            ins=[sin_d.opt()], outs=[sg_d.opt()])
        s8 = const.tile([1, NCORES], F32)
        DMA(s8[:], sg_d[:])
        st = const.tile([1, 1], F32)
        nc.vector.reduce_sum(st[:], s8[:], axis=mybir.AxisListType.X)
        logs = const.tile([1, 1], F32)
        nc.scalar.activation(logs[:], st[:], mybir.ActivationFunctionType.Ln)
        negls = const.tile([1, 1], F32)
        nc.vector.tensor_scalar_mul(negls[:], logs[:], -1.0)

        # ---- final normalize: out = logits - logS (alternate DVE/ACT)
        for j, (off, w) in enumerate(OUT_CHUNKS):
            lgin = fpool.tile([1, 512], F32, tag="lgin", name=f"lgin{j}", bufs=5)
            DMA(lgin[:, 0:w], lgr_d[:, off:off + w])
            outc = fpool.tile([1, 512], F32, tag="outc", name=f"outc{j}", bufs=5)
            if j % 2 == 0:
                nc.vector.tensor_scalar_add(outc[:, 0:w], lgin[:, 0:w], negls[:])
            else:
                nc.scalar.activation(outc[:, 0:w], lgin[:, 0:w],
                                     mybir.ActivationFunctionType.Identity,
                                     bias=negls[:], scale=1.0)
            DMA(t["logits_out"][:, off:off + w], outc[:, 0:w])


# ---------------------------------------------------------------- host prep
def shard_inputs(input, hidden, encoder_outputs, emb, attn_W, attn_b,
                 comb_W, comb_b, W_ih, W_hh, b_ih, b_hh, out_W, out_b):
    """Build the 8 per-core input maps (numpy)."""
    idx = int(np.asarray(input).reshape(-1)[0])
    embedded = np.asarray(emb[idx], dtype=np.float32)          # [300]
    h0 = np.asarray(hidden, dtype=np.float32).reshape(H)       # [2048]
    attn_W = np.asarray(attn_W, dtype=np.float32)
    attn_b = np.asarray(attn_b, dtype=np.float32)
    comb_W = np.asarray(comb_W, dtype=np.float32)
    comb_b = np.asarray(comb_b, dtype=np.float32)
    W_ih = np.asarray(W_ih, dtype=np.float32)
    W_hh = np.asarray(W_hh, dtype=np.float32)
    b_ih = np.asarray(b_ih, dtype=np.float32)
    b_hh = np.asarray(b_hh, dtype=np.float32)
    out_W = np.asarray(out_W, dtype=np.float32)
    out_b = np.asarray(out_b, dtype=np.float32)
    enc = np.asarray(encoder_outputs, dtype=np.float32)

    # cat1 (reordered): [h0; embedded; 1.0; zeros] in partition layout
    cat1 = np.zeros(KP, dtype=np.float32)
    cat1[0:H] = h0
    cat1[H:H + E] = embedded
    cat1[EH] = 1.0
    cat1_p = np.ascontiguousarray(cat1.reshape(NK, 128).T, dtype=_np(DT_ATTN))
    emb1_p = np.ascontiguousarray(
        cat1[H:].reshape(3, 128).T, dtype=_np(DT_COMB))
    h0g_p = np.ascontiguousarray(h0.reshape(16, 128).T, dtype=_np(DT_GRU))
    one = np.zeros((128, 1), dtype=np.float32)
    one[0, 0] = 1.0

    # attn_W columns reordered to [h-part; e-part], bias row appended
    awt = np.zeros((KP, L), dtype=np.float32)
    awt[0:H] = attn_W[:, E:EH].T
    awt[H:EH] = attn_W[:, 0:E].T
    awt[EH] = attn_b
    awt = awt.astype(_np(DT_ATTN))

    enc_c = np.ascontiguousarray(enc, dtype=_np(DT_ATTN))

    per_core = []
    for c in range(NCORES):
        rows = slice(c * HS, (c + 1) * HS)
        cwt = np.zeros((KP, HS), dtype=np.float32)
        cwt[0:H] = comb_W[rows, E:EH].T
        cwt[H:EH] = comb_W[rows, 0:E].T
        cwt[EH] = comb_b[rows]
        grows = np.concatenate(
            [np.arange(g * H + c * HS, g * H + (c + 1) * HS) for g in range(3)])
        wih_t = np.ascontiguousarray(W_ih[grows].T, dtype=_np(DT_GRU))
        whh_t = np.ascontiguousarray(W_hh[grows].T, dtype=_np(DT_GRU))
        vrows = slice(c * VS, (c + 1) * VS)
        owt = np.ascontiguousarray(out_W[vrows].T, dtype=_np(DT_OUT))
        per_core.append({
            "cat1_p": cat1_p,
            "emb1_p": emb1_p,
            "h0g_p": h0g_p,
            "one_g": one.astype(_np(DT_GRU)),
            "one_o": one.astype(_np(DT_OUT)),
            "h0f": h0[rows].reshape(1, HS).copy(),
            "attn_WT": awt,
            "enc": enc_c,
            "comb_WT": cwt.astype(_np(DT_COMB)),
            "wih_T": wih_t,
            "whh_T": whh_t,
            "gib": b_ih[grows].reshape(1, GS).astype(_np(DT_GRU)),
            "ghb": b_hh[grows].reshape(1, GS).astype(_np(DT_GRU)),
            "outWT": owt,
            "outb": out_b[vrows].reshape(1, VS).astype(_np(DT_OUT)),
        })
    return per_core


# ---------------------------------------------------------------- entry
_CACHED_NC = None
LAST_RESULT = None


def kernel(**inputs):
    global _CACHED_NC, LAST_RESULT
    try:
        import axon_profile_shim
        axon_profile_shim.install()
    except Exception:
        pass
    if _CACHED_NC is None:
        _CACHED_NC = build_nc()
    in_maps = shard_inputs(**inputs)
    trace = bool(int(__import__("os").environ.get("KERNEL_TRACE", "0")))
    res = run_bass_kernel_spmd(
        _CACHED_NC, in_maps, core_ids=list(range(NCORES)), trace=trace)
    LAST_RESULT = res
    logits = np.concatenate(
        [res.results[c]["logits_out"] for c in range(NCORES)], axis=1)
    hidden_new = res.results[0]["hidden_out"].reshape(1, 1, H)
    attn_weights = res.results[0]["attnw_out"].reshape(1, L)
    return (logits.astype(np.float32),
            hidden_new.astype(np.float32),
            attn_weights.astype(np.float32))


# revision 19
# speedup vs baseline: 1.2698x; 1.0755x over previous
"""Trainium2 Bass kernel for single-step AttnDecoderRNN (batch=1 decode).

Strategy (8-way tensor parallel, one NEFF, SPMD):
  - embedding gather happens on host (only the one needed row is shipped)
  - attention (attn_W, encoder_outputs) replicated: every core computes the
    full 512 attn weights and the full attended context (cheap: ~9MB)
  - attn_combine row-sharded over H  -> x_c [256]     -> AllGather -> x [2048]
  - GRU (W_ih, W_hh) row-sharded over gates/H -> h_c [256] -> AllGather -> h
  - out_W row-sharded over V: each core computes 6250 logits + local sum-exp
    -> AllGather of the 8 partial sums -> local log-softmax normalization
  - biases folded into the matmuls as extra contraction rows (rank-1 update
    with a one-hot stationary operand), so they cost ~nothing

All matvecs y = A @ v run on the PE as out[1, n] += lhsT.T @ rhs with
lhsT = v k-slice [128, 1] (stationary) and rhs = A.T tile [128, n<=512]
(moving), with A.T pre-transposed on the host so every DMA is contiguous.
Vectors produced in free-layout [1, N] are converted to partition-layout
[128, N/128] via a PE transpose against an identity matrix.
"""

import numpy as np
import ml_dtypes

import concourse.bacc as bacc
import concourse.mybir as mybir
import concourse.tile as tile
from concourse import masks
from concourse.bass_utils import run_bass_kernel_spmd

# ---------------------------------------------------------------- constants
V, E, H, L = 50000, 300, 2048, 512
EH = E + H                      # 2348
KP = 2432                       # EH padded to 19*128 (incl. bias row at 2348)
NK = KP // 128                  # 19
NCORES = 8
VS = V // NCORES                # 6250 logits per core
HS = H // NCORES                # 256 hidden per core
GS = 3 * HS                     # 768 gate rows per core
F32 = mybir.dt.float32
BF16 = mybir.dt.bfloat16

# dtype knobs per weight group (host cast + device compute dtype)
import os as _os
_KDT = _os.environ.get("KERNEL_DTYPES", "ffff")  # attn, comb, gru, out
DT_ATTN = BF16 if _KDT[0] == "b" else F32
DT_COMB = BF16 if _KDT[1] == "b" else F32
DT_GRU = BF16 if _KDT[2] == "b" else F32
DT_OUT = BF16 if _KDT[3] == "b" else F32

_NPDT = {F32: np.float32, BF16: ml_dtypes.bfloat16}

OUT_CHUNKS = [(j * 512, min(512, VS - j * 512)) for j in range((VS + 511) // 512)]


def _np(dt):
    return _NPDT[dt]


# ---------------------------------------------------------------- device IR
def build_nc():
    nc = bacc.Bacc(trn_type="TRN2", num_devices=NCORES, debug=False)

    def din(name, shape, dt):
        return nc.dram_tensor(name, shape, dt, kind="ExternalInput").ap()

    ident16 = din("ident16", [16, 16], F32)
    cat1_p = din("cat1_p", [128, NK], DT_ATTN)
    emb1_p = din("emb1_p", [128, 3], DT_COMB)
    h0g_p = din("h0g_p", [128, 16], DT_GRU)
    one_g = din("one_g", [128, 1], DT_GRU)
    one_o = din("one_o", [128, 1], DT_OUT)
    h0f = din("h0f", [1, HS], F32)
    attn_WT = din("attn_WT", [KP, L], DT_ATTN)
    enc = din("enc", [L, H], DT_ATTN)
    comb_WT = din("comb_WT", [KP, HS], DT_COMB)
    wih_T = din("wih_T", [H, GS], DT_GRU)
    whh_T = din("whh_T", [H, GS], DT_GRU)
    gib = din("gib", [1, GS], DT_GRU)
    ghb = din("ghb", [1, GS], DT_GRU)
    outWT = din("outWT", [H, VS], DT_OUT)
    outb = din("outb", [1, VS], DT_OUT)

    logits_out = nc.dram_tensor("logits_out", [1, VS], F32, kind="ExternalOutput").ap()
    hidden_out = nc.dram_tensor("hidden_out", [1, H], F32, kind="ExternalOutput").ap()
    attnw_out = nc.dram_tensor("attnw_out", [1, L], F32, kind="ExternalOutput").ap()
    if _os.environ.get("KERNEL_DEBUG"):
        dbg_cat2 = nc.dram_tensor("dbg_cat2", [128, NK], DT_COMB,
                                  kind="ExternalOutput").ap()
        dbg_x = nc.dram_tensor("dbg_x", [1, H], F32, kind="ExternalOutput").ap()

    with tile.TileContext(nc) as tc:
        _body(nc, tc, locals())
    nc.compile()
    return nc


def _body(nc, tc, t):
    MM = nc.tensor.matmul
    import contextlib
    import itertools
    ctx = contextlib.ExitStack()
    # round-robin bulk DMAs over the two HWDGE queues (SP + ACT)
    _bulk_cycle = itertools.cycle([nc.sync, nc.scalar])

    def BDMA(out, in_):
        next(_bulk_cycle).dma_start(out, in_)

    DMA = nc.sync.dma_start
    with ctx:
        const = ctx.enter_context(tc.tile_pool(name="const", bufs=1))
        bigw = ctx.enter_context(tc.tile_pool(name="bigw", bufs=2))
        owtp = ctx.enter_context(tc.tile_pool(name="owtp", bufs=20))
        fpool = ctx.enter_context(tc.tile_pool(name="fpool", bufs=2))
        ps_s = ctx.enter_context(tc.tile_pool(name="ps_s", bufs=3, space="PSUM"))
        ps_o = ctx.enter_context(tc.tile_pool(name="ps_o", bufs=5, space="PSUM"))
        dram = ctx.enter_context(tc.tile_pool(name="dram", bufs=1, space="DRAM"))

        def load_weight(name, src, kn, width, dt, pieces=2):
            """One SBUF tile [128, kn*width]; col-block k = src[k*128:(k+1)*128, :].
            DMA'd in `pieces` pieces for pipelining (subtile deps), spread
            over both HWDGE queues."""
            w = bigw.tile([128, kn * width], dt, tag="bigw", name=name)
            srcv = src.rearrange("(k p) n -> p k n", p=128)
            step = (kn + pieces - 1) // pieces
            for h0 in range(0, kn, step):
                h1 = min(h0 + step, kn)
                BDMA(w[:, h0 * width:h1 * width], srcv[:, h0:h1, :])
            return w

        # ---- constants / small inputs
        ident = const.tile([16, 16], F32)
        DMA(ident[:], t["ident16"])
        ca1 = const.tile([128, NK], DT_ATTN)
        DMA(ca1[:], t["cat1_p"])
        emb1 = const.tile([128, 3], DT_COMB)
        DMA(emb1[:], t["emb1_p"])
        h0g = const.tile([128, 16], DT_GRU)
        DMA(h0g[:], t["h0g_p"])
        oneg = const.tile([128, 1], DT_GRU)
        DMA(oneg[:], t["one_g"])
        oneo = const.tile([128, 1], DT_OUT)
        DMA(oneo[:], t["one_o"])
        h0f_t = const.tile([1, HS], F32)
        DMA(h0f_t[:], t["h0f"])
        gib_t = const.tile([1, GS], DT_GRU)
        DMA(gib_t[:], t["gib"])
        ghb_t = const.tile([1, GS], DT_GRU)
        DMA(ghb_t[:], t["ghb"])

        # ---- attention logits: al[1, 512] = cat1 @ attn_W.T (+attn_b via row 2348)
        awt = load_weight("awt", t["attn_WT"], NK, L, DT_ATTN, pieces=3)
        psum_al = ps_s.tile([1, 512], F32, tag="pss")
        for k in range(NK):
            MM(psum_al[:], lhsT=ca1[:, k:k + 1], rhs=awt[:, k * L:(k + 1) * L],
               start=(k == 0), stop=(k == NK - 1))

        # ---- softmax over 512 on partition 0
        mx = const.tile([1, 1], F32)
        nc.vector.reduce_max(mx[:], psum_al[:], axis=mybir.AxisListType.X)
        negm = const.tile([1, 1], F32)
        nc.vector.tensor_scalar_mul(negm[:], mx[:], -1.0)
        e_sb = const.tile([1, 512], F32)
        s1 = const.tile([1, 1], F32)
        nc.scalar.activation(e_sb[:], psum_al[:], mybir.ActivationFunctionType.Exp,
                             bias=negm[:], scale=1.0, accum_out=s1[:])
        rs = const.tile([1, 1], F32)
        nc.vector.reciprocal(rs[:], s1[:])
        aw_sb = const.tile([1, 512], F32)
        nc.vector.tensor_scalar_mul(aw_sb[:], e_sb[:], rs[:])
        DMA(t["attnw_out"], aw_sb[:])

        # ---- attn weights to partition layout [128, 4]
        aw4 = const.tile([4, 128], F32)
        for i in range(4):
            DMA(aw4[i:i + 1, :], aw_sb[:, i * 128:(i + 1) * 128])
        ps_awp = ps_s.tile([128, 4], F32, tag="pss")
        nc.tensor.transpose(ps_awp[:], aw4[:], ident[0:4, 0:4])
        wp = const.tile([128, 4], DT_ATTN)
        nc.vector.tensor_copy(wp[:], ps_awp[:])

        # ---- attended context directly in partition layout [128, 16]
        # NOTE: accumulation groups within one PSUM tile must be contiguous
        # (start=True clears bank-wide), so loop j-outer / k-inner.
        encw = load_weight("encw", t["enc"], 4, H, DT_ATTN, pieces=2)
        ps_aa = ps_s.tile([128, 16], F32, tag="pss")
        for j in range(16):
            for k in range(4):
                MM(ps_aa[:, j:j + 1],
                   lhsT=encw[:, k * H + j * 128:k * H + (j + 1) * 128],
                   rhs=wp[:, k:k + 1], start=(k == 0), stop=(k == 3))

        cat2 = const.tile([128, NK], DT_COMB)
        nc.vector.tensor_copy(cat2[:, 0:16], ps_aa[:])
        nc.scalar.copy(cat2[:, 16:19], emb1[:])
        if "dbg_cat2" in t:
            DMA(t["dbg_cat2"], cat2[:])

        # ---- attn_combine row-shard: x_c[1, 256] = relu(cat2 @ comb_W_c.T + b)
        cwt = load_weight("cwt", t["comb_WT"], NK, HS, DT_COMB, pieces=1)
        psum_x = ps_s.tile([1, 512], F32, tag="pss")
        for k in range(NK):
            MM(psum_x[:, 0:HS], lhsT=cat2[:, k:k + 1],
               rhs=cwt[:, k * HS:(k + 1) * HS],
               start=(k == 0), stop=(k == NK - 1))
        x_sb = const.tile([1, HS], F32)
        nc.scalar.activation(x_sb[:], psum_x[:, 0:HS],
                             mybir.ActivationFunctionType.Relu)

        # ---- GRU gh half FIRST (independent of x -> overlaps the AllGather)
        # gh = h0 @ W_hh_c.T + b_hh_c
        ps_gh_a = ps_s.tile([1, 512], F32, tag="pss")
        ps_gh_b = ps_s.tile([1, 512], F32, tag="pss")
        whw = load_weight("whw", t["whh_T"], 16, GS, DT_GRU, pieces=2)
        wiw = load_weight("wiw", t["wih_T"], 16, GS, DT_GRU, pieces=2)
        MM(ps_gh_a[:], lhsT=oneg[0:1, 0:1], rhs=ghb_t[:, 0:512],
           start=True, stop=False)
        MM(ps_gh_b[:, 0:HS], lhsT=oneg[0:1, 0:1], rhs=ghb_t[:, 512:768],
           start=True, stop=False)
        for k in range(16):
            MM(ps_gh_a[:], lhsT=h0g[:, k:k + 1], rhs=whw[:, k * GS:k * GS + 512],
               start=False, stop=(k == 15))
            MM(ps_gh_b[:, 0:HS], lhsT=h0g[:, k:k + 1],
               rhs=whw[:, k * GS + 512:(k + 1) * GS],
               start=False, stop=(k == 15))
        gha = const.tile([1, 512], F32)
        nc.scalar.copy(gha[:], ps_gh_a[:])
        ghb_sb = const.tile([1, HS], F32)
        nc.scalar.copy(ghb_sb[:], ps_gh_b[:, 0:HS])

        # ---- AllGather x -> [2048] -> partition layout
        xin_d = dram.tile([1, HS], F32)
        xg_d = dram.tile([1, H], F32)
        DMA(xin_d[:], x_sb[:])
        nc.gpsimd.collective_compute(
            "AllGather", mybir.AluOpType.bypass,
            replica_groups=[list(range(NCORES))],
            ins=[xin_d.opt()], outs=[xg_d.opt()])
        if "dbg_x" in t:
            DMA(t["dbg_x"], xg_d[:])
        xf = const.tile([16, 128], F32)
        DMA(xf[:], xg_d[:].rearrange("a (b c) -> (a b) c", c=128))
        ps_xp = ps_s.tile([128, 16], F32, tag="pss")
        nc.tensor.transpose(ps_xp[:], xf[:], ident[:])
        xp = const.tile([128, 16], DT_GRU)
        nc.vector.tensor_copy(xp[:], ps_xp[:])

        # ---- GRU gi half: gi = x @ W_ih_c.T + b_ih_c, then the gate math
        ps_gi_a = ps_s.tile([1, 512], F32, tag="pss")
        ps_gi_b = ps_s.tile([1, 512], F32, tag="pss")
        MM(ps_gi_a[:], lhsT=oneg[0:1, 0:1], rhs=gib_t[:, 0:512],
           start=True, stop=False)
        MM(ps_gi_b[:, 0:HS], lhsT=oneg[0:1, 0:1], rhs=gib_t[:, 512:768],
           start=True, stop=False)
        for k in range(16):
            MM(ps_gi_a[:], lhsT=xp[:, k:k + 1], rhs=wiw[:, k * GS:k * GS + 512],
               start=False, stop=(k == 15))
            MM(ps_gi_b[:, 0:HS], lhsT=xp[:, k:k + 1],
               rhs=wiw[:, k * GS + 512:(k + 1) * GS],
               start=False, stop=(k == 15))

        rzpre = const.tile([1, 512], F32)
        nc.vector.tensor_add(rzpre[:], ps_gi_a[:], gha[:])
        rz = const.tile([1, 512], F32)
        nc.scalar.activation(rz[:], rzpre[:], mybir.ActivationFunctionType.Sigmoid)
        rhn = const.tile([1, HS], F32)
        nc.vector.tensor_mul(rhn[:], rz[:, 0:HS], ghb_sb[:])
        npre = const.tile([1, HS], F32)
        nc.vector.tensor_add(npre[:], ps_gi_b[:, 0:HS], rhn[:])
        n_sb = const.tile([1, HS], F32)
        nc.scalar.activation(n_sb[:], npre[:], mybir.ActivationFunctionType.Tanh)
        dd = const.tile([1, HS], F32)
        nc.vector.tensor_sub(dd[:], h0f_t[:], n_sb[:])
        zd = const.tile([1, HS], F32)
        nc.vector.tensor_mul(zd[:], rz[:, HS:2 * HS], dd[:])
        hnew = const.tile([1, HS], F32)
        nc.vector.tensor_add(hnew[:], n_sb[:], zd[:])

        # ---- AllGather h -> [2048]; emit hidden output; partition layout
        hin_d = dram.tile([1, HS], F32)
        hg_d = dram.tile([1, H], F32)
        DMA(hin_d[:], hnew[:])
        nc.gpsimd.collective_compute(
            "AllGather", mybir.AluOpType.bypass,
            replica_groups=[list(range(NCORES))],
            ins=[hin_d.opt()], outs=[hg_d.opt()])
        DMA(t["hidden_out"], hg_d[:])
        hf = const.tile([16, 128], F32)
        DMA(hf[:], hg_d[:].rearrange("a (b c) -> (a b) c", c=128))
        ps_hp = ps_s.tile([128, 16], F32, tag="pss")
        nc.tensor.transpose(ps_hp[:], hf[:], ident[:])
        hp = const.tile([128, 16], DT_OUT)
        nc.vector.tensor_copy(hp[:], ps_hp[:])

        # ---- output projection row-shard: logits_c[6250] = h @ out_W_c.T + b_c
        # raw logits stay in SBUF; per-chunk exp+accum gives the local sum
        ssum = const.tile([1, len(OUT_CHUNKS)], F32)
        lg_sb = const.tile([1, VS], F32)
        GRP = 2048
        n_grp = (VS + GRP - 1) // GRP
        for g in range(n_grp):
            g0 = g * GRP
            gw = min(GRP, VS - g0)
            tiles_k = []
            for k in range(16):
                owt = owtp.tile([128, GRP], DT_OUT, tag="owt",
                                name=f"owt{g}_{k}")
                BDMA(owt[:, 0:gw],
                     t["outWT"][k * 128:(k + 1) * 128, g0:g0 + gw])
                tiles_k.append(owt)
            for jj in range((gw + 511) // 512):
                j = g * 4 + jj
                off, w = OUT_CHUNKS[j]
                ps = ps_o.tile([1, 512], F32, tag="po", name=f"po{j}")
                ob = fpool.tile([1, 512], DT_OUT, tag="ob", name=f"ob{j}")
                DMA(ob[:, 0:w], t["outb"][:, off:off + w])
                MM(ps[:, 0:w], lhsT=oneo[0:1, 0:1], rhs=ob[:, 0:w],
                   start=True, stop=False)
                for k in range(16):
                    MM(ps[:, 0:w], lhsT=hp[:, k:k + 1],
                       rhs=tiles_k[k][:, jj * 512:jj * 512 + w],
                       start=False, stop=(k == 15))
                nc.scalar.copy(lg_sb[:, off:off + w], ps[:, 0:w])
                esc = fpool.tile([1, 512], F32, tag="esc", name=f"esc{j}")
                nc.scalar.activation(esc[:, 0:w], ps[:, 0:w],
                                     mybir.ActivationFunctionType.Exp,
                                     accum_out=ssum[:, j:j + 1])

        # ---- global log-sum-exp via AllGather of the 8 local sums
        sl = const.tile([1, 1], F32)
        nc.vector.reduce_sum(sl[:], ssum[:], axis=mybir.AxisListType.X)
        sin_d = dram.tile([1, 1], F32)
        sg_d = dram.tile([1, NCORES], F32)
        DMA(sin_d[:], sl[:])
        nc.gpsimd.collective_compute(
            "AllGather", mybir.AluOpType.bypass,
            replica_groups=[list(range(NCORES))],
            ins=[sin_d.opt()], outs=[sg_d.opt()])
        s8 = const.tile([1, NCORES], F32)
        DMA(s8[:], sg_d[:])
        st = const.tile([1, 1], F32)
        nc.vector.reduce_sum(st[:], s8[:], axis=mybir.AxisListType.X)
        logs = const.tile([1, 1], F32)
        nc.scalar.activation(logs[:], st[:], mybir.ActivationFunctionType.Ln)
        negls = const.tile([1, 1], F32)
        nc.vector.tensor_scalar_mul(negls[:], logs[:], -1.0)

        # ---- final normalize in place, split across DVE / ACT
        HALF = 3072
        nc.vector.tensor_scalar_add(lg_sb[:, 0:HALF], lg_sb[:, 0:HALF], negls[:])
        nc.scalar.activation(lg_sb[:, HALF:VS], lg_sb[:, HALF:VS],
                             mybir.ActivationFunctionType.Identity,
                             bias=negls[:], scale=1.0)
        DMA(t["logits_out"][:, 0:HALF], lg_sb[:, 0:HALF])
        nc.scalar.dma_start(t["logits_out"][:, HALF:VS], lg_sb[:, HALF:VS])


# ---------------------------------------------------------------- host prep
def shard_inputs(input, hidden, encoder_outputs, emb, attn_W, attn_b,
                 comb_W, comb_b, W_ih, W_hh, b_ih, b_hh, out_W, out_b):
    """Build the 8 per-core input maps (numpy)."""
    idx = int(np.asarray(input).reshape(-1)[0])
    embedded = np.asarray(emb[idx], dtype=np.float32)          # [300]
    h0 = np.asarray(hidden, dtype=np.float32).reshape(H)       # [2048]
    attn_W = np.asarray(attn_W, dtype=np.float32)
    attn_b = np.asarray(attn_b, dtype=np.float32)
    comb_W = np.asarray(comb_W, dtype=np.float32)
    comb_b = np.asarray(comb_b, dtype=np.float32)
    W_ih = np.asarray(W_ih, dtype=np.float32)
    W_hh = np.asarray(W_hh, dtype=np.float32)
    b_ih = np.asarray(b_ih, dtype=np.float32)
    b_hh = np.asarray(b_hh, dtype=np.float32)
    out_W = np.asarray(out_W, dtype=np.float32)
    out_b = np.asarray(out_b, dtype=np.float32)
    enc = np.asarray(encoder_outputs, dtype=np.float32)

    # cat1 (reordered): [h0; embedded; 1.0; zeros] in partition layout
    cat1 = np.zeros(KP, dtype=np.float32)
    cat1[0:H] = h0
    cat1[H:H + E] = embedded
    cat1[EH] = 1.0
    cat1_p = np.ascontiguousarray(cat1.reshape(NK, 128).T, dtype=_np(DT_ATTN))
    emb1_p = np.ascontiguousarray(
        cat1[H:].reshape(3, 128).T, dtype=_np(DT_COMB))
    h0g_p = np.ascontiguousarray(h0.reshape(16, 128).T, dtype=_np(DT_GRU))
    one = np.zeros((128, 1), dtype=np.float32)
    one[0, 0] = 1.0

    # attn_W columns reordered to [h-part; e-part], bias row appended
    awt = np.zeros((KP, L), dtype=np.float32)
    awt[0:H] = attn_W[:, E:EH].T
    awt[H:EH] = attn_W[:, 0:E].T
    awt[EH] = attn_b
    awt = awt.astype(_np(DT_ATTN))

    enc_c = np.ascontiguousarray(enc, dtype=_np(DT_ATTN))

    per_core = []
    for c in range(NCORES):
        rows = slice(c * HS, (c + 1) * HS)
        cwt = np.zeros((KP, HS), dtype=np.float32)
        cwt[0:H] = comb_W[rows, E:EH].T
        cwt[H:EH] = comb_W[rows, 0:E].T
        cwt[EH] = comb_b[rows]
        grows = np.concatenate(
            [np.arange(g * H + c * HS, g * H + (c + 1) * HS) for g in range(3)])
        wih_t = np.ascontiguousarray(W_ih[grows].T, dtype=_np(DT_GRU))
        whh_t = np.ascontiguousarray(W_hh[grows].T, dtype=_np(DT_GRU))
        vrows = slice(c * VS, (c + 1) * VS)
        owt = np.ascontiguousarray(out_W[vrows].T, dtype=_np(DT_OUT))
        per_core.append({
            "ident16": np.eye(16, dtype=np.float32),
            "cat1_p": cat1_p,
            "emb1_p": emb1_p,
            "h0g_p": h0g_p,
            "one_g": one.astype(_np(DT_GRU)),
            "one_o": one.astype(_np(DT_OUT)),
            "h0f": h0[rows].reshape(1, HS).copy(),
            "attn_WT": awt,
            "enc": enc_c,
            "comb_WT": cwt.astype(_np(DT_COMB)),
            "wih_T": wih_t,
            "whh_T": whh_t,
            "gib": b_ih[grows].reshape(1, GS).astype(_np(DT_GRU)),
            "ghb": b_hh[grows].reshape(1, GS).astype(_np(DT_GRU)),
            "outWT": owt,
            "outb": out_b[vrows].reshape(1, VS).astype(_np(DT_OUT)),
        })
    return per_core


# ---------------------------------------------------------------- entry
_CACHED_NC = None
LAST_RESULT = None


def kernel(**inputs):
    global _CACHED_NC, LAST_RESULT
    try:
        import axon_profile_shim
        axon_profile_shim.install()
    except Exception:
        pass
    if _CACHED_NC is None:
        _CACHED_NC = build_nc()
    in_maps = shard_inputs(**inputs)
    trace = bool(int(__import__("os").environ.get("KERNEL_TRACE", "0")))
    res = run_bass_kernel_spmd(
        _CACHED_NC, in_maps, core_ids=list(range(NCORES)), trace=trace)
    LAST_RESULT = res
    logits = np.concatenate(
        [res.results[c]["logits_out"] for c in range(NCORES)], axis=1)
    hidden_new = res.results[0]["hidden_out"].reshape(1, 1, H)
    attn_weights = res.results[0]["attnw_out"].reshape(1, L)
    return (logits.astype(np.float32),
            hidden_new.astype(np.float32),
            attn_weights.astype(np.float32))
